# revision 6
# baseline (speedup 1.0000x reference)
"""KAN layer (B=8192, IN_F=OUT_F=1024, GRID=5) on 8 Trainium2 cores.

Math: Y[b,o] = W0[o]*silu(x) + spline_o(clip(x,-1,1)) + b[o], x = X[b,o].
The degree-1 B-spline is evaluated in the *segment* basis
    spline(clip(x)) = A''[o] + sum_j gamma_j[o] * v_j(x),
    v_j(x) = clip(x, s_{j-1}, s_j),  knots s = (-1,-0.5,0,0.5,1),
    gamma_j = w1 * m_j (segment slopes),
so each map is a 2-op tensor_scalar clip straight from x.

Sharding: edges across the 8 cores (128 edges/core, full batch 8192 on the
free dim).  Per core TensorE does a per-edge diagonal combine of 5 feature
maps into PSUM: v2,v3 ride ONE fp8e4 DoubleRow matmul (2 maps/pass), v1,v4
and silu are fp16 matmuls.  ScalarE: silu + most of the PSUM evacuation
(Identity+bias); VectorE: the 4 clips + the evac remainder.  I/O is fp16
(host converts); fp8 weight-quantization error is minimax-compensated into
the per-edge bias on host.
"""
import sys

for _p in ("/root/.axon_site", "/root/.axon_site/_ro/trn_rl_repo", "/root/.axon_site/_ro/pypackages"):
    if _p not in sys.path:
        sys.path.append(_p)

import numpy as np
import ml_dtypes

import concourse.bacc as bacc
import concourse.tile as tile
from concourse import mybir
from concourse.bass_utils import run_bass_kernel_spmd

B, IN_F, OUT_F, GRID = 8192, 1024, 1024, 5
N_CORES = 8
PER = OUT_F // N_CORES          # 128 edges per core
NB = B                          # 8192 batch columns per core
SBLK = 2048                     # superblock columns
NSB = NB // SBLK                # 4 superblocks
CHUNK = 512                     # one PSUM bank of fp32
SPLIT = 1920                    # evac columns done on ScalarE (rest on VectorE)

_nc_cache = None


def _build():
    f32 = mybir.dt.float32
    f16 = mybir.dt.float16
    f8 = mybir.dt.float8e4
    AF = mybir.ActivationFunctionType
    OP = mybir.AluOpType
    DRm = mybir.MatmulPerfMode.DoubleRow

    nc = bacc.Bacc("TRN2", target_bir_lowering=False, debug=False)
    xt = nc.dram_tensor("xt", [PER, NB], f16, kind="ExternalInput").ap()
    cpack = nc.dram_tensor("cpack", [PER, 8], f32, kind="ExternalInput").ap()
    ident = nc.dram_tensor("ident", [PER, 128], f16, kind="ExternalInput").ap()
    yt = nc.dram_tensor("yt", [PER, NB], f16, kind="ExternalOutput").ap()

    with tile.TileContext(nc) as tc:
        with tc.tile_pool(name="const", bufs=1) as cpool, \
             tc.tile_pool(name="xin", bufs=1) as xpool, \
             tc.tile_pool(name="sil", bufs=2) as spool, \
             tc.tile_pool(name="v14", bufs=2) as vpool, \
             tc.tile_pool(name="v23", bufs=2) as wpool, \
             tc.tile_pool(name="yout", bufs=2) as ypool, \
             tc.tile_pool(name="ps", bufs=2, space="PSUM") as pspool:
            # input loads first on the fast HWDGE/sync ring; small first chunk
            # so compute can start as early as possible
            x0 = xpool.tile([128, SBLK], f16, tag="x0", name="x0")
            nc.sync.dma_start(x0[:, 0:1024], xt[:, 0:1024])
            nc.sync.dma_start(x0[:, 1024:SBLK], xt[:, 1024:SBLK])
            x1 = xpool.tile([128, SBLK], f16, tag="x1", name="x1")
            nc.sync.dma_start(x1[:], xt[:, SBLK:2 * SBLK])
            x23 = xpool.tile([128, 2 * SBLK], f16, tag="x23", name="x23")
            nc.sync.dma_start(x23[:], xt[:, 2 * SBLK:4 * SBLK])

            # consts on the gpsimd/SWDGE ring (parallel with the x loads)
            cp = cpool.tile([128, 8], f32)
            nc.gpsimd.dma_start(cp[:], cpack[:, :])
            id16 = cpool.tile([128, 128], f16)
            nc.gpsimd.dma_start(id16[:], ident[:, :])

            scr = cpool.tile([128, CHUNK], f16)
            nc.vector.memset(scr[:], 0.25)
            # silu ACT-table load overlaps the first input DMA
            dum = cpool.tile([128, 1], f16)
            nc.scalar.activation(dum[:], scr[:, 0:1], AF.Silu)

            # PE warm-up: ~4.3us of matmuls on scratch so HAM reaches 8/8
            pswarm = pspool.tile([128, SBLK], f32, tag="ps", name="pswarm")
            for r in range(10):
                nc.tensor.matmul(pswarm[:, 0:CHUNK], scr[:, 0:128], scr[:],
                                 start=True, stop=True, skip_group_check=True)

            # per-edge diagonal stationaries (on-device from ident * weight)
            dsil = cpool.tile([128, 128], f16)
            nc.vector.tensor_scalar_mul(dsil[:], id16[:], cp[:, 0:1])
            dv1 = cpool.tile([128, 128], f16)
            nc.vector.tensor_scalar_mul(dv1[:], id16[:], cp[:, 1:2])
            dp23 = cpool.tile([128, 2, 128], f8)
            nc.vector.tensor_scalar_mul(dp23[:, 0, :], id16[:], cp[:, 2:3])
            nc.vector.tensor_scalar_mul(dp23[:, 1, :], id16[:], cp[:, 3:4])
            dv4 = cpool.tile([128, 128], f16)
            nc.vector.tensor_scalar_mul(dv4[:], id16[:], cp[:, 4:5])

            sil23 = None
            for j in range(NSB):
                if j == 0:
                    xv = x0[:]
                elif j == 1:
                    xv = x1[:]
                else:
                    xv = x23[:, (j - 2) * SBLK:(j - 1) * SBLK]

                if j < 2:
                    sil = spool.tile([128, SBLK], f16, tag="sil", name=f"sil{j}")
                else:
                    # one FD=4096 silu covers SB2+SB3 (amortizes ACT overhead)
                    if sil23 is None:
                        sil23 = spool.tile([128, 2 * SBLK], f16, tag="sil23",
                                           name="sil23")
                        nc.scalar.activation(sil23[:], x23[:], AF.Silu)
                    sil = sil23[:, (j - 2) * SBLK:(j - 1) * SBLK]

                v23 = wpool.tile([128, 2, SBLK], f8, tag="v23", name=f"v23_{j}")
                v1 = vpool.tile([128, SBLK], f16, tag="v1", name=f"v1_{j}")
                v4 = vpool.tile([128, SBLK], f16, tag="v4", name=f"v4_{j}")
                ps = pspool.tile([128, SBLK], f32, tag="ps", name=f"ps{j}")
                y = ypool.tile([128, SBLK], f16, tag="y", name=f"y{j}")

                halves = (0, 1) if j in (0, NSB - 1) else (None,)
                for h in halves:
                    if h is None:
                        cl = slice(0, SBLK)
                        lo, hi = 0, 4
                    else:
                        cl = slice(h * 1024, (h + 1) * 1024)
                        lo, hi = 2 * h, 2 * h + 2
                    if j == 0:
                        sl = spool.tile([128, 1024], f16, tag=f"sil0{h}",
                                        name=f"sil0{h}")
                        nc.scalar.activation(sl[:], x0[:, cl], AF.Silu)
                        sil_ap = sl[:]
                        sil_lo = lo
                    elif h is None:
                        nc.scalar.activation(sil[:], xv, AF.Silu)
                        sil_ap = sil[:]
                        sil_lo = 0
                    else:
                        sil_ap = sil[:, cl]
                        sil_lo = lo
                    nc.vector.tensor_scalar(v23[:, 0, cl], xv[:, cl],
                                            0.0, -0.5, OP.min, OP.max)
                    nc.vector.tensor_scalar(v23[:, 1, cl], xv[:, cl],
                                            0.5, 0.0, OP.min, OP.max)
                    nc.vector.tensor_scalar(v1[:, cl], xv[:, cl],
                                            -0.5, -1.0, OP.min, OP.max)
                    nc.vector.tensor_scalar(v4[:, cl], xv[:, cl],
                                            1.0, 0.5, OP.min, OP.max)

                    # matmuls for this half (or whole superblock)
                    for c in range(lo, hi):
                        nc.tensor.matmul(ps[:, c * CHUNK:(c + 1) * CHUNK],
                                         dp23[:, 0:2, :],
                                         v23[:, 0:2, c * CHUNK:(c + 1) * CHUNK],
                                         start=True, stop=False, perf_mode=DRm,
                                         skip_group_check=True)
                    for c in range(lo, hi):
                        nc.tensor.matmul(ps[:, c * CHUNK:(c + 1) * CHUNK], dv1[:],
                                         v1[:, c * CHUNK:(c + 1) * CHUNK],
                                         start=False, stop=False,
                                         skip_group_check=True)
                    for c in range(lo, hi):
                        nc.tensor.matmul(ps[:, c * CHUNK:(c + 1) * CHUNK], dv4[:],
                                         v4[:, c * CHUNK:(c + 1) * CHUNK],
                                         start=False, stop=False,
                                         skip_group_check=True)
                    for c in range(lo, hi):
                        nc.tensor.matmul(
                            ps[:, c * CHUNK:(c + 1) * CHUNK], dsil[:],
                            sil_ap[:, (c - sil_lo) * CHUNK:(c - sil_lo + 1) * CHUNK],
                            start=False, stop=True, skip_group_check=True)

                    if h is not None:
                        # per-half evacuation (fast ramp on SB0, short tail on SB3)
                        a, b = h * 1024, (h + 1) * 1024
                        sp = a + 960
                        nc.scalar.activation(y[:, a:sp], ps[:, a:sp], AF.Identity,
                                             bias=cp[:, 5:6], scale=1.0)
                        nc.vector.tensor_scalar(y[:, sp:b], ps[:, sp:b],
                                                cp[:, 5:6], None, OP.add)

                if halves == (None,):
                    nc.scalar.activation(y[:, 0:SPLIT], ps[:, 0:SPLIT], AF.Identity,
                                         bias=cp[:, 5:6], scale=1.0)
                    nc.vector.tensor_scalar(y[:, SPLIT:SBLK], ps[:, SPLIT:SBLK],
                                            cp[:, 5:6], None, OP.add)

                if j < NSB - 1:
                    nc.sync.dma_start(yt[:, j * SBLK:(j + 1) * SBLK], y[:])
                else:
                    nc.sync.dma_start(yt[:, j * SBLK:j * SBLK + 1024], y[:, 0:1024])
                    nc.sync.dma_start(yt[:, j * SBLK + 1024:(j + 1) * SBLK],
                                      y[:, 1024:SBLK])
    nc.compile()
    return nc


def _host_prep(X, coeffs, W, b):
    """Per-core cpack [128, 8] fp32: W0, g1, g2, g3, g4, A'' (compensated)."""
    c = coeffs.astype(np.float64)
    W64 = W.astype(np.float64)
    b64 = b.astype(np.float64)
    m = 2.0 * (c[:, 1:] - c[:, :-1])          # [O, 4] segment slopes
    w1 = W64[:, 1]
    gam = w1[:, None] * m                      # [O, 4]
    s = np.array([-1.0, -0.5, 0.0, 0.5])
    A = b64 + w1 * c[:, 0] - (gam * s[None, :]).sum(1)
    # minimax compensation of fp8e4 quantization of g2, g3 (device uses RNE)
    d2 = gam[:, 1].astype(ml_dtypes.float8_e4m3).astype(np.float64) - gam[:, 1]
    d3 = gam[:, 2].astype(ml_dtypes.float8_e4m3).astype(np.float64) - gam[:, 2]
    cand = np.stack([-0.5 * d2, np.zeros_like(d2), 0.5 * d3], 1)
    A = A - (cand.max(1) + cand.min(1)) / 2

    cpack = np.zeros((OUT_F, 8), dtype=np.float32)
    cpack[:, 0] = W64[:, 0]
    cpack[:, 1:5] = gam
    cpack[:, 5] = A
    return cpack


def kernel(X, coeffs, W, b):
    global _nc_cache
    if _nc_cache is None:
        _nc_cache = _build()
    nc = _nc_cache

    cpack = _host_prep(X, coeffs, W, b)
    ident = np.eye(128, dtype=np.float16)
    X16 = X.astype(np.float16)
    in_maps = []
    for cidx in range(N_CORES):
        sl = slice(cidx * PER, (cidx + 1) * PER)
        in_maps.append({
            "xt": np.ascontiguousarray(X16[:, sl].T),
            "cpack": np.ascontiguousarray(cpack[sl]),
            "ident": ident,
        })

    res = run_bass_kernel_spmd(nc, in_maps, core_ids=list(range(N_CORES)))
    Y = np.empty((B, OUT_F), dtype=np.float32)
    for cidx in range(N_CORES):
        sl = slice(cidx * PER, (cidx + 1) * PER)
        Y[:, sl] = res.results[cidx]["yt"].T.astype(np.float32)
    return Y
